# revision 35
# baseline (speedup 1.0000x reference)
"""Trainium2 Bass kernel for Qwen3-Next GatedDeltaNet (4096 tokens, 2048 hidden,
16 k-heads / 32 v-heads x 128 dims).

Sharding: tensor-parallel over v-heads across 8 cores (4 v-heads = 2 k-heads per
core).  Each core computes its qkvz/ba projection shard, runs the chunked gated
delta rule (chunk C=128) for its heads, and produces a partial out-projection
[2048, 4096] (transposed).  The host sums the 8 partials and transposes.

v2 restructure vs baseline:
  - state chain in matrix form: S_{n+1} = (gend I - K''^T Wt) S_n + K''^T U0,
    with -(Wt^T K'') and K'' chunk-parallel precomputed; the sequential part
    per chunk-head is 3 matmuls + one PSUM->SBUF copy.
  - output: psO = gm^T U0 (precompute) + QP^T S_n (chain), QP = q~T - Wt^T gm.
  - per-token scales folded into colform products; engines rebalanced
    (nothing hot on gpsimd, scalar/vector split).
  - phases pipelined: proj(h1) overlaps rec(h0); out-proj(h0) overlaps rec(h1).
    PSUM statically budgeted to 8 banks.
"""

import os
import sys
from contextlib import ExitStack

for _p in ("/opt/trn_rl_repo", "/root/.axon_site/_ro/trn_rl_repo"):
    if os.path.isdir(_p) and _p not in sys.path:
        sys.path.append(_p)

import numpy as np

import concourse.bass as bass
import concourse.mybir as mybir
import concourse.tile as tile
from concourse import bacc
from concourse.masks import make_identity
from concourse.bass import ds, ts

AFT = mybir.ActivationFunctionType
ALU = mybir.AluOpType
F32 = mybir.dt.float32
BF16 = mybir.dt.bfloat16

L = 4096
H = 2048
DK = 128
DV = 128
NCORES = 8
KH = 2
VH = 4
QKVZ_SH = 1536
BA_SH = 8
C = 128
NCHUNK = L // C       # 32
HCHUNK = NCHUNK // 2  # 16 chunks per half
HL = L // 2           # 2048 tokens per half
EPS = 1e-6
LN128 = float(np.log(128.0))


# n-tile kinds within the 1536-wide shard (12 tiles of 128 cols)
def tile_kind(n):
    m = n % 6
    grp = n // 6
    if m == 0:
        return ("q", grp)
    if m == 1:
        return ("k", grp)
    if m in (2, 3):
        return ("v", grp * 2 + (m - 2))
    return ("z", grp * 2 + (m - 4))


def prepend_bcast(ap: bass.AP, n: int = 128) -> bass.AP:
    return bass.AP(tensor=ap.tensor, offset=ap.offset, ap=[[0, n]] + list(ap.ap))


def build_kernel(nc: bass.Bass, tc: "tile.TileContext"):
    # ---------------- I/O ----------------
    hidden = nc.dram_tensor("hidden", [L, H], F32, kind="ExternalInput").ap()
    wqkvz = nc.dram_tensor("wqkvz", [H, QKVZ_SH], F32, kind="ExternalInput").ap()
    wba = nc.dram_tensor("wba", [H, BA_SH], F32, kind="ExternalInput").ap()
    alog = nc.dram_tensor("alog", [1, VH], F32, kind="ExternalInput").ap()
    dtb = nc.dram_tensor("dtb", [1, VH], F32, kind="ExternalInput").ap()
    nw = nc.dram_tensor("nw", [1, DV], F32, kind="ExternalInput").ap()
    wout = nc.dram_tensor("wout", [VH * DV, H], F32, kind="ExternalInput").ap()
    out = nc.dram_tensor("out", [H, L], BF16, kind="ExternalOutput").ap()

    ctx = ExitStack()
    const = ctx.enter_context(tc.tile_pool(name="const", bufs=1))
    dram = ctx.enter_context(tc.tile_pool(name="dram", bufs=1, space="DRAM"))
    colp = ctx.enter_context(tc.tile_pool(name="colp", bufs=1))
    big = ctx.enter_context(tc.tile_pool(name="big", bufs=1))

    # psum pools: exactly 8 banks total
    psP = ctx.enter_context(tc.tile_pool(name="psP", bufs=2, space="PSUM"))   # 2
    psK = ctx.enter_context(tc.tile_pool(name="psK", bufs=1, space="PSUM"))   # 1
    psZ = ctx.enter_context(tc.tile_pool(name="psZ", bufs=1, space="PSUM"))   # 1
    psPC = ctx.enter_context(tc.tile_pool(name="psPC", bufs=1, space="PSUM"))  # 1
    psO = ctx.enter_context(tc.tile_pool(name="psO", bufs=2, space="PSUM"))   # 2
    psS = ctx.enter_context(tc.tile_pool(name="psS", bufs=1, space="PSUM"))   # 1

    # DRAM scratch
    wq16d = dram.tile([H, QKVZ_SH], BF16, tag="wq16d")
    qrowD = dram.tile([NCHUNK, 128, KH, 128], BF16, tag="qrowD")
    krowD = dram.tile([NCHUNK, 128, KH, 128], BF16, tag="krowD")
    vrowD = dram.tile([NCHUNK, 128, VH, 128], BF16, tag="vrowD")
    szD = dram.tile([NCHUNK, 128, VH, 128], BF16, tag="szD")
    browsD = dram.tile([2 * VH, L], F32, tag="browsD")  # row 2vh=chat, 2vh+1=ctil
    cendD = dram.tile([1, NCHUNK * VH], F32, tag="cendD")

    # ---------------- constants ----------------
    ident = const.tile([128, 128], F32, tag="ident")
    make_identity(nc, ident)
    ident_bf = const.tile([128, 128], BF16, tag="ident_bf")
    make_identity(nc, ident_bf)

    # additive exponent masks: kept entries 0, masked entries -50000 (exp -> 0)
    # block0 (atn / KK): keep strict-upper (j<i); block1 (gm / KQ): keep incl-upper
    maskADD = const.tile([128, 2, 128], F32, tag="maskADD")
    nc.gpsimd.memset(maskADD[:, 0, :], -50000.0)
    nc.gpsimd.affine_select(
        out=maskADD[:, 0, :], in_=maskADD[:, 0, :],
        compare_op=ALU.is_ge, fill=0.0, base=0,
        pattern=[[-1, 128]], channel_multiplier=1,
    )
    nc.gpsimd.memset(maskADD[:, 1, :], -50000.0)
    nc.gpsimd.affine_select(
        out=maskADD[:, 1, :], in_=maskADD[:, 1, :],
        compare_op=ALU.is_gt, fill=0.0, base=0,
        pattern=[[-1, 128]], channel_multiplier=1,
    )

    uincl = const.tile([128, 128], F32, tag="uincl")  # U[t,j]=1 if t<=j
    nc.gpsimd.memset(uincl, 0.0)
    nc.gpsimd.affine_select(
        out=uincl, in_=uincl,
        compare_op=ALU.is_gt, fill=1.0, base=0,
        pattern=[[-1, 128]], channel_multiplier=1,
    )
    nwz = const.tile([128, 128], F32, tag="nwz")
    nc.sync.dma_start(out=nwz, in_=prepend_bcast(nw[0:1, :]))
    dtb_b = const.tile([128, 1, VH], F32, tag="dtb_b")
    nc.sync.dma_start(out=dtb_b, in_=prepend_bcast(dtb[0:1, :]))
    negea_b = const.tile([128, 1, VH], F32, tag="negea_b")
    nc.sync.dma_start(out=negea_b, in_=prepend_bcast(alog[0:1, :]))
    nc.scalar.activation(negea_b, negea_b, AFT.Exp)
    nc.vector.tensor_scalar_mul(negea_b, negea_b, -1.0)
    c_eps = const.tile([128, 1], F32, tag="c_eps")
    nc.vector.memset(c_eps, EPS)

    # ---------------- weights prep (cast DMAs) ----------------
    nc.gpsimd.dma_start(out=wq16d, in_=wqkvz)
    wout_bf = big.tile([128, VH, H], BF16, tag="wout_bf")
    for i in range(VH):
        nc.gpsimd.dma_start(out=wout_bf[:, i, :], in_=wout[ts(i, 128), :])
    wba_bf = const.tile([128, H // 128, BA_SH], BF16, tag="wba_bf")
    nc.gpsimd.dma_start(out=wba_bf, in_=wba.rearrange("(i p) c -> p i c", p=128))

    # ---------------- colform tiles ----------------
    def cf(name, w=VH):
        return colp.tile([128, NCHUNK, w], F32, tag=name, name=name)

    bcol = cf("bcol")
    acol = cf("acol")
    beta_col = cf("beta_col")
    lnb_col = cf("lnb_col")
    g_col = cf("g_col")
    c_col = cf("c_col")
    cendb = cf("cendb")
    gend_col = cf("gend_col")
    gam_col = cf("gam_col")
    gi_col = cf("gi_col")
    sR1_col = cf("sR1_col")     # beta*gamma*rk
    sK_col = cf("sK_col")       # rk*gi
    ogq_col = cf("ogq_col")     # gamma*rq
    bek_col = cf("bek_col")     # lnrk - c  (e12 bias)
    sscol = cf("sscol")
    rstdc = cf("rstdc")
    normq = cf("normq", KH)
    normk = cf("normk", KH)
    lnrk_col = cf("lnrk_col", KH)
    lnrq_col = cf("lnrq_col", KH)
    rkv_col = cf("rkv_col", KH)
    rqv_col = cf("rqv_col", KH)
    bro_col = cf("bro_col", 2 * VH)   # cols (vh,2): chat, ctil
    t1_col = cf("t1_col")

    # ---------------- big SBUF tiles ----------------
    # dim-major raw k: [128 dims, half, kh, HL tokens]
    kdT = big.tile([128, 2, KH, HL], BF16, tag="kdT")

    # staging pools
    stH = ctx.enter_context(tc.tile_pool(name="stH", bufs=2))
    stW = ctx.enter_context(tc.tile_pool(name="stW", bufs=2))
    stE = ctx.enter_context(tc.tile_pool(name="stE", bufs=2))
    stR = ctx.enter_context(tc.tile_pool(name="stR", bufs=2))
    stZl = ctx.enter_context(tc.tile_pool(name="stZl", bufs=2))
    work = ctx.enter_context(tc.tile_pool(name="work", bufs=2))
    spool = ctx.enter_context(tc.tile_pool(name="spool", bufs=2))
    hT_pool = ctx.enter_context(tc.tile_pool(name="hTp", bufs=1))

    # S state: [128 dk, vh, DV]
    S4 = spool.tile([128, VH, DV], BF16, tag="S4")
    nc.gpsimd.memset(S4, 0.0)
    S4c = [S4]

    ev_rot = [0]

    def evict_copy(dst, src, alternate=True):
        ev_rot[0] += 1
        if alternate and ev_rot[0] % 2 == 0:
            nc.scalar.activation(dst, src, AFT.Copy)
        else:
            nc.vector.tensor_copy(dst, src)

    # =============== PROJ PHASE (per half) ===============
    def emit_proj_dmas(half):
        hT = hT_pool.tile([128, H // 128, HL], BF16, tag="hT", name="hT")
        for t in range(HL // 128):
            hbf = stH.tile([128, H], BF16, tag="hbf")
            nc.gpsimd.dma_start(out=hbf, in_=hidden[ds(half * HL + t * 128, 128), :])
            nc.sync.dma_start(out=hT[:, :, ts(t, 128)], in_=hbf, transpose=True)
        return hT

    def emit_ba_proj(half, hT):

        # ---- ba projection -> bcol/acol ----
        for s in range(HL // 512):
            pba = psP.tile([BA_SH, 512], F32, tag="pp")
            for i in range(H // 128):
                nc.tensor.matmul(pba, wba_bf[:, i, :], hT[:, i, ts(s, 512)],
                                 start=(i == 0), stop=(i == H // 128 - 1))
            sb8 = stE.tile([BA_SH, 512], F32, tag="sb8", bufs=1)
            nc.vector.tensor_copy(sb8, pba)
            for c4 in range(4):
                ng = half * HCHUNK + s * 4 + c4
                tpb = psP.tile([128, BA_SH], F32, tag="pp")
                nc.tensor.transpose(tpb, sb8[:, ts(c4, 128)], ident[:BA_SH, :BA_SH])
                nc.vector.tensor_copy(bcol[:, ng, :], tpb[:, 0:VH])
                nc.vector.tensor_copy(acol[:, ng, :], tpb[:, VH:BA_SH])

    # ---- qkvz projection: one (n, q) unit ----
    wt_cur = {}

    def emit_proj_unit(half, hT, n, q):
        kind, idx = tile_kind(n)
        if q == 0:
            wt = stW.tile([128, H // 128, 128], BF16, tag="wt", bufs=2,
                          name="wt")
            nc.scalar.dma_start(
                out=wt,
                in_=wq16d.rearrange("(i p) c -> p i c", p=128)[:, :, ts(n, 128)])
            wt_cur[0] = wt
        wt = wt_cur[0]
        if True:
            if True:
                pp = psP.tile([128, 512], F32, tag="pp")
                for i in range(H // 128):
                    nc.tensor.matmul(pp, wt[:, i, :], hT[:, i, ts(q, 512)],
                                     start=(i == 0), stop=(i == H // 128 - 1))
                ch0 = half * HCHUNK + q * 4
                if kind == "k":
                    dst = kdT[:, half, idx, ts(q, 512)]
                    evict_copy(dst, pp)
                    rowst = stR.tile([128, 4, 128], BF16, tag="rowst")
                    nc.scalar.dma_start(out=rowst, in_=dst, transpose=True)
                    scr = stR.tile([128, 4, 128], BF16, tag="nscr")
                    for b4 in range(4):
                        nc.scalar.activation(scr[:, b4, :], rowst[:, b4, :],
                                             AFT.Square,
                                             accum_out=normk[:, ch0 + b4, idx:idx + 1])
                    nc.scalar.dma_start(
                        out=bass.AP(
                            tensor=krowD.tensor,
                            offset=krowD.offset + ch0 * 128 * KH * 128 + idx * 128,
                            ap=[[KH * 128, 128], [128 * KH * 128, 4], [1, 128]]),
                        in_=rowst)
                elif kind == "q":
                    ev = stE.tile([128, 512], BF16, tag="ev")
                    evict_copy(ev, pp)
                    rowst = stR.tile([128, 4, 128], BF16, tag="rowst")
                    nc.scalar.dma_start(out=rowst, in_=ev, transpose=True)
                    scr = stR.tile([128, 4, 128], BF16, tag="nscr")
                    for b4 in range(4):
                        nc.scalar.activation(scr[:, b4, :], rowst[:, b4, :],
                                             AFT.Square,
                                             accum_out=normq[:, ch0 + b4, idx:idx + 1])
                    nc.scalar.dma_start(
                        out=bass.AP(
                            tensor=qrowD.tensor,
                            offset=qrowD.offset + ch0 * 128 * KH * 128 + idx * 128,
                            ap=[[KH * 128, 128], [128 * KH * 128, 4], [1, 128]]),
                        in_=rowst)
                elif kind == "v":
                    ev = stE.tile([128, 512], BF16, tag="ev")
                    evict_copy(ev, pp)
                    rowst = stR.tile([128, 4, 128], BF16, tag="rowst")
                    nc.scalar.dma_start(out=rowst, in_=ev, transpose=True)
                    nc.scalar.dma_start(
                        out=bass.AP(
                            tensor=vrowD.tensor,
                            offset=vrowD.offset + ch0 * 128 * VH * 128 + idx * 128,
                            ap=[[VH * 128, 128], [128 * VH * 128, 4], [1, 128]]),
                        in_=rowst)
                else:  # z -> silu -> szD
                    ev = stE.tile([128, 512], BF16, tag="ev")
                    evict_copy(ev, pp)
                    zrow = stZl.tile([128, 4, 128], BF16, tag="zrow")
                    nc.scalar.dma_start(out=zrow, in_=ev, transpose=True)
                    sgm = stZl.tile([128, 4, 128], BF16, tag="sgm")
                    nc.scalar.activation(sgm, zrow, AFT.Sigmoid)
                    zn = stZl.tile([128, 4, 128], BF16, tag="zn")
                    nc.vector.tensor_tensor(
                        zn, zrow, nwz[:, None, :].to_broadcast((128, 4, 128)),
                        op=ALU.mult)
                    szt = stZl.tile([128, 4, 128], BF16, tag="szt")
                    nc.vector.tensor_tensor(szt, zn, sgm, op=ALU.mult)
                    nc.scalar.dma_start(
                        out=bass.AP(
                            tensor=szD.tensor,
                            offset=szD.offset + ch0 * 128 * VH * 128 + idx * 128,
                            ap=[[VH * 128, 128], [128 * VH * 128, 4], [1, 128]]),
                        in_=szt)

    # =============== COLFORM PHASE (per half) ===============
    def emit_colform(half):
        hs = ds(half * HCHUNK, HCHUNK)
        nc.scalar.activation(beta_col[:, hs, :], bcol[:, hs, :], AFT.Sigmoid)
        nc.scalar.activation(lnb_col[:, hs, :], beta_col[:, hs, :], AFT.Ln)
        nc.vector.tensor_tensor(g_col[:, hs, :], acol[:, hs, :],
                                dtb_b.to_broadcast((128, HCHUNK, VH)), op=ALU.add)
        nc.scalar.activation(g_col[:, hs, :], g_col[:, hs, :], AFT.Exp)
        nc.scalar.activation(g_col[:, hs, :], g_col[:, hs, :], AFT.Ln, bias=1.0)
        nc.vector.tensor_tensor(g_col[:, hs, :], g_col[:, hs, :],
                                negea_b.to_broadcast((128, HCHUNK, VH)), op=ALU.mult)
        for j in range(HCHUNK):
            n = half * HCHUNK + j
            pc = psP.tile([128, VH], F32, tag="pp")
            nc.tensor.matmul(pc, uincl, g_col[:, n, :], start=True, stop=True)
            nc.vector.tensor_copy(c_col[:, n, :], pc)
        nc.scalar.activation(gam_col[:, hs, :], c_col[:, hs, :], AFT.Exp)
        nc.sync.dma_start(out=cendD[:, ds(half * HCHUNK * VH, HCHUNK * VH)],
                          in_=c_col[127:128, hs, :].rearrange("p a b -> p (a b)"))
        nc.sync.dma_start(
            out=cendb[:, hs, :],
            in_=prepend_bcast(cendD[0:1, ds(half * HCHUNK * VH, HCHUNK * VH)]
                              .rearrange("o (a b) -> o a b", b=VH)))
        nc.scalar.activation(gend_col[:, hs, :], cendb[:, hs, :], AFT.Exp)
        nc.vector.tensor_tensor(gi_col[:, hs, :], cendb[:, hs, :], c_col[:, hs, :],
                                op=ALU.subtract)
        nc.scalar.activation(gi_col[:, hs, :], gi_col[:, hs, :], AFT.Exp)
        # norm scales
        nc.scalar.activation(lnrk_col[:, hs, :], normk[:, hs, :], AFT.Ln, bias=c_eps)
        nc.vector.tensor_scalar_mul(lnrk_col[:, hs, :], lnrk_col[:, hs, :], -0.5)
        nc.scalar.activation(rkv_col[:, hs, :], lnrk_col[:, hs, :], AFT.Exp)
        nc.scalar.activation(lnrq_col[:, hs, :], normq[:, hs, :], AFT.Ln, bias=c_eps)
        nc.vector.tensor_scalar(lnrq_col[:, hs, :], lnrq_col[:, hs, :], -0.5,
                                -0.5 * LN128, op0=ALU.mult, op1=ALU.add)
        nc.scalar.activation(rqv_col[:, hs, :], lnrq_col[:, hs, :], AFT.Exp)
        # products
        nc.vector.tensor_tensor(t1_col[:, hs, :], beta_col[:, hs, :],
                                gam_col[:, hs, :], op=ALU.mult)  # beta*gamma
        for kh in range(KH):
            vs = ds(kh * 2, 2)
            rk_b = rkv_col[:, hs, kh:kh + 1].to_broadcast((128, HCHUNK, 2))
            rq_b = rqv_col[:, hs, kh:kh + 1].to_broadcast((128, HCHUNK, 2))
            nc.vector.tensor_tensor(sR1_col[:, hs, vs], t1_col[:, hs, vs], rk_b,
                                    op=ALU.mult)
            nc.vector.tensor_tensor(sK_col[:, hs, vs], gi_col[:, hs, vs], rk_b,
                                    op=ALU.mult)
            nc.vector.tensor_tensor(ogq_col[:, hs, vs], gam_col[:, hs, vs], rq_b,
                                    op=ALU.mult)
            lnrk_b = lnrk_col[:, hs, kh:kh + 1].to_broadcast((128, HCHUNK, 2))
            nc.vector.tensor_tensor(bek_col[:, hs, vs], lnrk_b, c_col[:, hs, vs],
                                    op=ALU.subtract)
        # chat/ctil rows
        nc.vector.tensor_tensor(t1_col[:, hs, :], c_col[:, hs, :],
                                lnb_col[:, hs, :], op=ALU.add)  # c + ln(beta)
        for vh in range(VH):
            kh = vh // 2
            nc.vector.tensor_tensor(bro_col[:, hs, 2 * vh], t1_col[:, hs, vh],
                                    lnrk_col[:, hs, kh], op=ALU.add)
            nc.vector.tensor_tensor(bro_col[:, hs, 2 * vh + 1], c_col[:, hs, vh],
                                    lnrq_col[:, hs, kh], op=ALU.add)
        # transpose bro -> browsD (per-chunk writes)
        for j in range(HCHUNK):
            n = half * HCHUNK + j
            pbr = psP.tile([2 * VH, 128], F32, tag="pp")
            nc.tensor.transpose(pbr, bro_col[:, n, :], ident)
            sbr = work.tile([2 * VH, 128], F32, tag="sbr")
            nc.vector.tensor_copy(sbr, pbr)
            nc.sync.dma_start(out=browsD[:, ts(n, 128)], in_=sbr)

    # =============== REC PHASE (per half) ===============
    xth = {}

    def emit_rec(half, tail_hooks=None, proj_units=None):
        xth[half] = hT_pool.tile([128, VH, HL], BF16, tag="xTh", name="xTh")

        for j in range(HCHUNK):
            n = half * HCHUNK + j
            csl = ds(j * 128, 128)
            kD = work.tile([128, KH, 128], BF16, tag="kD", bufs=2)
            nc.sync.dma_start(out=kD, in_=krowD[n])
            vD = work.tile([128, VH, 128], BF16, tag="vD", bufs=2)
            nc.sync.dma_start(out=vD, in_=vrowD[n])
            qD = work.tile([128, KH, 128], BF16, tag="qD", bufs=2)
            nc.sync.dma_start(out=qD, in_=qrowD[n])
            szv = work.tile([128, VH, 128], BF16, tag="szv", bufs=2)
            nc.sync.dma_start(out=szv, in_=szD[n])
            qTc = work.tile([128, KH, 128], BF16, tag="qTc", bufs=2)
            nc.sync.dma_start(out=qTc, in_=qD, transpose=True)
            cb8 = work.tile([128, 2 * VH, 128], F32, tag="cb8", bufs=2)
            nc.gpsimd.dma_start(out=cb8, in_=prepend_bcast(
                browsD[:, ds(n * 128, 128)]))
            chb = [cb8[:, ds(2 * vh, 2), :] for vh in range(VH)]

            # q~ rows + transpose to dim-major
            qt_r = work.tile([128, VH, 128], BF16, tag="qt_r")
            for vh in range(VH):
                nc.vector.tensor_scalar(qt_r[:, vh, :], qD[:, vh // 2, :],
                                        ogq_col[:, n, vh:vh + 1], None,
                                        op0=ALU.mult)
            qtT = work.tile([128, VH, 128], BF16, tag="qtT")
            nc.scalar.dma_start(out=qtT, in_=qt_r, transpose=True)

            ag_t = [[None, None], [None, None]]
            Z4_t = []
            K4_t = []
            for kh in range(KH):
                psk = psK.tile([128, 2, 128], F32, tag="kkq")
                kT = kdT[:, half, kh, csl]
                nc.tensor.matmul(psk[:, 0, :], kT, kT, start=True, stop=True)
                nc.tensor.matmul(psk[:, 1, :], kT, qTc[:, kh, :],
                                 start=False, stop=True)

                for vi in range(2):
                    vh = kh * 2 + vi
                    # masked exponent: kept entries chat-ish, masked -> -5e4
                    r12 = work.tile([128, 2, 128], F32, tag="r12")
                    nc.vector.tensor_tensor(r12, chb[vh], maskADD, op=ALU.add)
                    # e12 = exp(r12 + lnrk_j - c_j)  (positive A | Ghat)
                    e12 = work.tile([128, 2, 128], F32, tag="e12")
                    nc.scalar.activation(e12, r12, AFT.Exp,
                                         bias=bek_col[:, n, vh:vh + 1])
                    ag = work.tile([128, 2, 128], BF16, tag="ag", bufs=6)
                    nc.vector.tensor_tensor(ag, e12, psk, op=ALU.mult)
                    ag_t[kh][vi] = ag

                # solve: Z = R - A^T R  (ag block0 = +A stored [j,i])
                R4 = work.tile([128, 2, 2, 128], BF16, tag="R4")
                for vi in range(2):
                    vh = kh * 2 + vi
                    nc.vector.tensor_scalar(R4[:, vi, 0, :], vD[:, vh, :],
                                            beta_col[:, n, vh:vh + 1], None,
                                            op0=ALU.mult)
                    nc.vector.tensor_scalar(R4[:, vi, 1, :], kD[:, kh, :],
                                            sR1_col[:, n, vh:vh + 1], None,
                                            op0=ALU.mult)
                psz = psZ.tile([128, 2, 256], F32, tag="pz")
                for vi in range(2):
                    nc.tensor.matmul(
                        psz[:, vi, :], ag_t[kh][vi][:, 0, :],
                        R4[:, vi, :, :].rearrange("p a b -> p (a b)"),
                        start=(vi == 0), stop=True)
                Z4 = work.tile([128, 2, 2, 128], BF16, tag="Z4", bufs=3)
                nc.vector.tensor_tensor(
                    Z4.rearrange("p a b c -> p (a b c)"),
                    R4.rearrange("p a b c -> p (a b c)"),
                    psz.rearrange("p a b -> p (a b)"), op=ALU.subtract)
                Z4_t.append(Z4)

                K4 = work.tile([128, 2, 128], BF16, tag="K4", bufs=3)
                for vi in range(2):
                    vh = kh * 2 + vi
                    nc.vector.tensor_scalar(K4[:, vi, :], kD[:, kh, :],
                                            sK_col[:, n, vh:vh + 1], None,
                                            op0=ALU.mult)
                K4_t.append(K4)

            # P / CM precompute + psO precompute
            QP_t = []
            Mst_t = []
            pso = psO.tile([128, VH, 128], F32, tag="psO")
            first_o = True
            for kh in range(KH):
                ppc = psPC.tile([128, 2, 2, 128], F32, tag="pc")
                for vi in range(2):
                    Wt = Z4_t[kh][:, vi, 1, :]
                    nc.tensor.matmul(ppc[:, vi, 0, :], Wt, ag_t[kh][vi][:, 1, :],
                                     start=(vi == 0), stop=False)
                    nc.tensor.matmul(ppc[:, vi, 1, :], Wt, K4_t[kh][:, vi, :],
                                     start=False, stop=True)
                QP2 = work.tile([128, 2, 128], BF16, tag="QP2", bufs=3)
                nc.vector.tensor_tensor(QP2, qtT[:, ds(kh * 2, 2), :],
                                        ppc[:, :, 0, :], op=ALU.subtract)
                Mst2 = work.tile([128, 2, 128], BF16, tag="Mst2", bufs=3)
                nc.scalar.activation(Mst2, ppc[:, :, 1, :], AFT.Copy, scale=-1.0)
                QP_t.append(QP2)
                Mst_t.append(Mst2)
                for vi in range(2):
                    vh = kh * 2 + vi
                    nc.tensor.matmul(pso[:, vh, :], ag_t[kh][vi][:, 1, :],
                                     Z4_t[kh][:, vi, 0, :],
                                     start=first_o, stop=False)
                    first_o = False

            # ---- sequential chain ----
            Sold = S4c[0]
            Sg4 = work.tile([128, VH, 128], BF16, tag="Sg4")
            for vh in range(VH):
                nc.vector.tensor_scalar(Sg4[:, vh, :], Sold[:, vh, :],
                                        gend_col[:, n, vh:vh + 1], None,
                                        op0=ALU.mult)
            pss = psS.tile([128, VH, 128], F32, tag="psS")
            for vh in range(VH):
                kh, vi = vh // 2, vh % 2
                nc.tensor.matmul(pss[:, vh, :], Mst_t[kh][:, vi, :],
                                 Sold[:, vh, :], start=(vh == 0), stop=False)
                nc.tensor.matmul(pss[:, vh, :], K4_t[kh][:, vi, :],
                                 Z4_t[kh][:, vi, 0, :], start=False, stop=True)
                nc.tensor.matmul(pso[:, vh, :], QP_t[kh][:, vi, :],
                                 Sold[:, vh, :], start=False, stop=True)
            Snew = spool.tile([128, VH, DV], BF16, tag="S4")
            nc.vector.tensor_tensor(
                Snew.rearrange("p a b -> p (a b)"),
                pss.rearrange("p a b -> p (a b)"),
                Sg4.rearrange("p a b -> p (a b)"), op=ALU.add)
            S4c[0] = Snew

            # ---- x output ----
            sqd = work.tile([128, 128], BF16, tag="sqd")
            for vh in range(VH):
                nc.scalar.activation(sqd, pso[:, vh, :], AFT.Square,
                                     accum_out=sscol[:, n, vh:vh + 1])
            rtmp = work.tile([128, VH], F32, tag="rtmp")
            nc.vector.tensor_scalar(rtmp, sscol[:, n, :], 1.0 / DV, EPS,
                                    op0=ALU.mult, op1=ALU.add)
            rtmp2 = work.tile([128, VH], F32, tag="rtmp2")
            nc.vector.reciprocal(rtmp2, rtmp)
            nc.scalar.activation(rstdc[:, n, :], rtmp2, AFT.Sqrt)
            szr = work.tile([128, VH, 128], BF16, tag="szr")
            xr = work.tile([128, VH, 128], BF16, tag="xr")
            for vh in range(VH):
                nc.scalar.activation(szr[:, vh, :], szv[:, vh, :], AFT.Copy,
                                     scale=rstdc[:, n, vh:vh + 1])
            nc.vector.tensor_tensor(
                xr.rearrange("p a b -> p (a b)"),
                pso.rearrange("p a b -> p (a b)"),
                szr.rearrange("p a b -> p (a b)"), op=ALU.mult)
            nc.scalar.dma_start(out=xth[half][:, :, ts(j, 128)], in_=xr,
                              transpose=True)
            if tail_hooks and j in tail_hooks:
                th_half, th_sg = tail_hooks[j]
                emit_tail_group(th_half, th_sg)
            if proj_units:
                for _ in range(3):
                    if proj_units:
                        proj_units.pop(0)()

    # =============== TAIL (out-projection, one 512-token group) ===============
    def emit_tail_group(half, sg):
        for nt in range(H // 128):
            po = psP.tile([128, 512], F32, tag="pp")
            for i in range(VH):
                nc.tensor.matmul(
                    po, wout_bf[:, i, ts(nt, 128)],
                    xth[half][:, i, ds(sg * 512, 512)],
                    start=(i == 0), stop=(i == VH - 1))
            ev = stE.tile([128, 512], BF16, tag="outev", bufs=2)
            evict_copy(ev, po, alternate=True)
            nc.gpsimd.dma_start(
                out=out[ts(nt, 128), ds(half * 2048 + sg * 512, 512)],
                in_=ev)

    # =============== emission order (pipeline priority) ===============
    hT0 = emit_proj_dmas(0)
    emit_ba_proj(0, hT0)
    for n in range(12):
        for q in range(4):
            emit_proj_unit(0, hT0, n, q)
    emit_colform(0)
    hT1 = emit_proj_dmas(1)
    units1 = [
        (lambda: emit_ba_proj(1, hT1))
    ] + [
        (lambda n=n, q=q: emit_proj_unit(1, hT1, n, q))
        for n in range(12) for q in range(4)
    ]
    emit_rec(0, proj_units=units1)
    while units1:
        units1.pop(0)()
    emit_colform(1)
    for sg in range(4):
        emit_tail_group(0, sg)
    hooks = {9: (1, 0), 11: (1, 1), 13: (1, 2)}
    emit_rec(1, tail_hooks=hooks)
    emit_tail_group(1, 3)

    ctx.close()
    return nc


_CACHED = None


def _build():
    global _CACHED
    if _CACHED is not None:
        return _CACHED
    nc = bacc.Bacc("TRN2", target_bir_lowering=False, debug=False)
    with tile.TileContext(nc) as tc:
        build_kernel(nc, tc)
    nc.compile()
    _CACHED = nc
    return nc


def make_in_maps(inputs):
    hidden = np.ascontiguousarray(np.asarray(inputs["hidden_states"], np.float32))
    W_qkvz = np.asarray(inputs["W_qkvz"], np.float32)
    W_ba = np.asarray(inputs["W_ba"], np.float32)
    A_log = np.asarray(inputs["A_log"], np.float32)
    dt_bias = np.asarray(inputs["dt_bias"], np.float32)
    norm_w = np.asarray(inputs["norm_weight"], np.float32)
    W_out = np.asarray(inputs["W_out"], np.float32)
    in_maps = []
    for c in range(NCORES):
        wba_sh = W_ba[:, c * BA_SH:(c + 1) * BA_SH]
        wba_r = wba_sh[:, [0, 1, 4, 5, 2, 3, 6, 7]]
        in_maps.append({
            "hidden": hidden,
            "wqkvz": np.ascontiguousarray(W_qkvz[:, c * QKVZ_SH:(c + 1) * QKVZ_SH]),
            "wba": np.ascontiguousarray(wba_r),
            "alog": np.ascontiguousarray(A_log[c * VH:(c + 1) * VH].reshape(1, VH)),
            "dtb": np.ascontiguousarray(dt_bias[c * VH:(c + 1) * VH].reshape(1, VH)),
            "nw": np.ascontiguousarray(norm_w.reshape(1, DV)),
            "wout": np.ascontiguousarray(W_out[c * VH * DV:(c + 1) * VH * DV, :]),
        })
    return in_maps


def kernel(**inputs) -> np.ndarray:
    from concourse import bass_utils

    nc = _build()
    in_maps = make_in_maps(inputs)
    res = bass_utils.run_bass_kernel_spmd(nc, in_maps, core_ids=list(range(NCORES)))
    total = None
    for r in res.results:
        o = np.asarray(r["out"], np.float32)
        total = o if total is None else total + o
    return np.ascontiguousarray(total.T)


# revision 36
# speedup vs baseline: 1.1005x; 1.1005x over previous
"""Trainium2 Bass kernel for Qwen3-Next GatedDeltaNet (4096 tokens, 2048 hidden,
16 k-heads / 32 v-heads x 128 dims).

Sharding: tensor-parallel over v-heads across 8 cores (4 v-heads = 2 k-heads per
core).  Each core computes its qkvz/ba projection shard, runs the chunked gated
delta rule (chunk C=128) for its heads, and produces a partial out-projection
[2048, 4096] (transposed).  The host sums the 8 partials and transposes.

v2 restructure vs baseline:
  - state chain in matrix form: S_{n+1} = (gend I - K''^T Wt) S_n + K''^T U0,
    with -(Wt^T K'') and K'' chunk-parallel precomputed; the sequential part
    per chunk-head is 3 matmuls + one PSUM->SBUF copy.
  - output: psO = gm^T U0 (precompute) + QP^T S_n (chain), QP = q~T - Wt^T gm.
  - per-token scales folded into colform products; engines rebalanced
    (nothing hot on gpsimd, scalar/vector split).
  - phases pipelined: proj(h1) overlaps rec(h0); out-proj(h0) overlaps rec(h1).
    PSUM statically budgeted to 8 banks.
"""

import os
import sys
from contextlib import ExitStack

for _p in ("/opt/trn_rl_repo", "/root/.axon_site/_ro/trn_rl_repo"):
    if os.path.isdir(_p) and _p not in sys.path:
        sys.path.append(_p)

import numpy as np

import concourse.bass as bass
import concourse.mybir as mybir
import concourse.tile as tile
from concourse import bacc
from concourse.masks import make_identity
from concourse.bass import ds, ts

AFT = mybir.ActivationFunctionType
ALU = mybir.AluOpType
F32 = mybir.dt.float32
BF16 = mybir.dt.bfloat16

L = 4096
H = 2048
DK = 128
DV = 128
NCORES = 8
KH = 2
VH = 4
QKVZ_SH = 1536
BA_SH = 8
C = 128
NCHUNK = L // C       # 32
HCHUNK = NCHUNK // 2  # 16 chunks per half
HL = L // 2           # 2048 tokens per half
EPS = 1e-6
LN128 = float(np.log(128.0))


# n-tile kinds within the 1536-wide shard (12 tiles of 128 cols)
def tile_kind(n):
    m = n % 6
    grp = n // 6
    if m == 0:
        return ("q", grp)
    if m == 1:
        return ("k", grp)
    if m in (2, 3):
        return ("v", grp * 2 + (m - 2))
    return ("z", grp * 2 + (m - 4))


def prepend_bcast(ap: bass.AP, n: int = 128) -> bass.AP:
    return bass.AP(tensor=ap.tensor, offset=ap.offset, ap=[[0, n]] + list(ap.ap))


def build_kernel(nc: bass.Bass, tc: "tile.TileContext"):
    # ---------------- I/O ----------------
    hidden = nc.dram_tensor("hidden", [L, H], F32, kind="ExternalInput").ap()
    wqkvz = nc.dram_tensor("wqkvz", [H, QKVZ_SH], F32, kind="ExternalInput").ap()
    wba = nc.dram_tensor("wba", [H, BA_SH], F32, kind="ExternalInput").ap()
    alog = nc.dram_tensor("alog", [1, VH], F32, kind="ExternalInput").ap()
    dtb = nc.dram_tensor("dtb", [1, VH], F32, kind="ExternalInput").ap()
    nw = nc.dram_tensor("nw", [1, DV], F32, kind="ExternalInput").ap()
    wout = nc.dram_tensor("wout", [VH * DV, H], F32, kind="ExternalInput").ap()
    out = nc.dram_tensor("out", [H, L], BF16, kind="ExternalOutput").ap()

    ctx = ExitStack()
    const = ctx.enter_context(tc.tile_pool(name="const", bufs=1))
    dram = ctx.enter_context(tc.tile_pool(name="dram", bufs=1, space="DRAM"))
    colp = ctx.enter_context(tc.tile_pool(name="colp", bufs=1))
    big = ctx.enter_context(tc.tile_pool(name="big", bufs=1))

    # psum pools: exactly 8 banks total
    psP = ctx.enter_context(tc.tile_pool(name="psP", bufs=2, space="PSUM"))   # 2
    psK = ctx.enter_context(tc.tile_pool(name="psK", bufs=1, space="PSUM"))   # 1
    psZ = ctx.enter_context(tc.tile_pool(name="psZ", bufs=1, space="PSUM"))   # 1
    psPC = ctx.enter_context(tc.tile_pool(name="psPC", bufs=1, space="PSUM"))  # 1
    psO = ctx.enter_context(tc.tile_pool(name="psO", bufs=2, space="PSUM"))   # 2
    psS = ctx.enter_context(tc.tile_pool(name="psS", bufs=1, space="PSUM"))   # 1

    # DRAM scratch
    wq16d = dram.tile([H, QKVZ_SH], BF16, tag="wq16d")
    qrowD = dram.tile([NCHUNK, 128, KH, 128], BF16, tag="qrowD")
    krowD = dram.tile([NCHUNK, 128, KH, 128], BF16, tag="krowD")
    vrowD = dram.tile([NCHUNK, 128, VH, 128], BF16, tag="vrowD")
    szD = dram.tile([NCHUNK, 128, VH, 128], BF16, tag="szD")
    browsD = dram.tile([2 * VH, L], F32, tag="browsD")  # row 2vh=chat, 2vh+1=ctil
    cendD = dram.tile([1, NCHUNK * VH], F32, tag="cendD")

    # ---------------- constants ----------------
    ident = const.tile([128, 128], F32, tag="ident")
    make_identity(nc, ident)
    ident_bf = const.tile([128, 128], BF16, tag="ident_bf")
    make_identity(nc, ident_bf)

    # additive exponent masks: kept entries 0, masked entries -50000 (exp -> 0)
    # block0 (atn / KK): keep strict-upper (j<i); block1 (gm / KQ): keep incl-upper
    maskADD = const.tile([128, 2, 128], F32, tag="maskADD")
    nc.gpsimd.memset(maskADD[:, 0, :], -50000.0)
    nc.gpsimd.affine_select(
        out=maskADD[:, 0, :], in_=maskADD[:, 0, :],
        compare_op=ALU.is_ge, fill=0.0, base=0,
        pattern=[[-1, 128]], channel_multiplier=1,
    )
    nc.gpsimd.memset(maskADD[:, 1, :], -50000.0)
    nc.gpsimd.affine_select(
        out=maskADD[:, 1, :], in_=maskADD[:, 1, :],
        compare_op=ALU.is_gt, fill=0.0, base=0,
        pattern=[[-1, 128]], channel_multiplier=1,
    )

    uincl = const.tile([128, 128], F32, tag="uincl")  # U[t,j]=1 if t<=j
    nc.gpsimd.memset(uincl, 0.0)
    nc.gpsimd.affine_select(
        out=uincl, in_=uincl,
        compare_op=ALU.is_gt, fill=1.0, base=0,
        pattern=[[-1, 128]], channel_multiplier=1,
    )
    nwz = const.tile([128, 128], F32, tag="nwz")
    nc.sync.dma_start(out=nwz, in_=prepend_bcast(nw[0:1, :]))
    dtb_b = const.tile([128, 1, VH], F32, tag="dtb_b")
    nc.sync.dma_start(out=dtb_b, in_=prepend_bcast(dtb[0:1, :]))
    negea_b = const.tile([128, 1, VH], F32, tag="negea_b")
    nc.sync.dma_start(out=negea_b, in_=prepend_bcast(alog[0:1, :]))
    nc.scalar.activation(negea_b, negea_b, AFT.Exp)
    nc.vector.tensor_scalar_mul(negea_b, negea_b, -1.0)
    c_eps = const.tile([128, 1], F32, tag="c_eps")
    nc.vector.memset(c_eps, EPS)

    # ---------------- weights prep (cast DMAs) ----------------
    nc.gpsimd.dma_start(out=wq16d, in_=wqkvz)
    wout_bf = big.tile([128, VH, H], BF16, tag="wout_bf")
    for i in range(VH):
        nc.gpsimd.dma_start(out=wout_bf[:, i, :], in_=wout[ts(i, 128), :])
    wba_bf = const.tile([128, H // 128, BA_SH], BF16, tag="wba_bf")
    nc.gpsimd.dma_start(out=wba_bf, in_=wba.rearrange("(i p) c -> p i c", p=128))

    # ---------------- colform tiles ----------------
    def cf(name, w=VH):
        return colp.tile([128, NCHUNK, w], F32, tag=name, name=name)

    bcol = cf("bcol")
    acol = cf("acol")
    beta_col = cf("beta_col")
    lnb_col = cf("lnb_col")
    g_col = cf("g_col")
    c_col = cf("c_col")
    cendb = cf("cendb")
    gend_col = cf("gend_col")
    gam_col = cf("gam_col")
    gi_col = cf("gi_col")
    sR1_col = cf("sR1_col")     # beta*gamma*rk
    sK_col = cf("sK_col")       # rk*gi
    ogq_col = cf("ogq_col")     # gamma*rq
    bek_col = cf("bek_col")     # lnrk - c  (e12 bias)
    sscol = cf("sscol")
    rstdc = cf("rstdc")
    normq = cf("normq", KH)
    normk = cf("normk", KH)
    lnrk_col = cf("lnrk_col", KH)
    lnrq_col = cf("lnrq_col", KH)
    rkv_col = cf("rkv_col", KH)
    rqv_col = cf("rqv_col", KH)
    bro_col = cf("bro_col", 2 * VH)   # cols (vh,2): chat, ctil
    t1_col = cf("t1_col")

    # ---------------- big SBUF tiles ----------------
    # dim-major raw k: [128 dims, half, kh, HL tokens]
    kdT = big.tile([128, 2, KH, HL], BF16, tag="kdT")

    # staging pools
    stH = ctx.enter_context(tc.tile_pool(name="stH", bufs=2))
    stW = ctx.enter_context(tc.tile_pool(name="stW", bufs=2))
    stE = ctx.enter_context(tc.tile_pool(name="stE", bufs=2))
    stR = ctx.enter_context(tc.tile_pool(name="stR", bufs=2))
    stZl = ctx.enter_context(tc.tile_pool(name="stZl", bufs=2))
    work = ctx.enter_context(tc.tile_pool(name="work", bufs=2))
    spool = ctx.enter_context(tc.tile_pool(name="spool", bufs=2))
    hT_pool = ctx.enter_context(tc.tile_pool(name="hTp", bufs=1))

    # S state: [128 dk, vh, DV]
    S4 = spool.tile([128, VH, DV], BF16, tag="S4")
    nc.gpsimd.memset(S4, 0.0)
    S4c = [S4]

    ev_rot = [0]

    def evict_copy(dst, src, alternate=True):
        ev_rot[0] += 1
        if alternate and ev_rot[0] % 2 == 0:
            nc.scalar.activation(dst, src, AFT.Copy)
        else:
            nc.vector.tensor_copy(dst, src)

    # =============== PROJ PHASE (per half) ===============
    def emit_proj_dmas(half):
        hT = hT_pool.tile([128, H // 128, HL], BF16, tag="hT", name="hT")
        for t in range(HL // 128):
            hbf = stH.tile([128, H], BF16, tag="hbf")
            nc.gpsimd.dma_start(out=hbf, in_=hidden[ds(half * HL + t * 128, 128), :])
            nc.sync.dma_start(out=hT[:, :, ts(t, 128)], in_=hbf, transpose=True)
        return hT

    def emit_ba_proj(half, hT):

        # ---- ba projection -> bcol/acol ----
        for s in range(HL // 512):
            pba = psP.tile([BA_SH, 512], F32, tag="pp")
            for i in range(H // 128):
                nc.tensor.matmul(pba, wba_bf[:, i, :], hT[:, i, ts(s, 512)],
                                 start=(i == 0), stop=(i == H // 128 - 1))
            sb8 = stE.tile([BA_SH, 512], F32, tag="sb8", bufs=1)
            nc.vector.tensor_copy(sb8, pba)
            for c4 in range(4):
                ng = half * HCHUNK + s * 4 + c4
                tpb = psP.tile([128, BA_SH], F32, tag="pp")
                nc.tensor.transpose(tpb, sb8[:, ts(c4, 128)], ident[:BA_SH, :BA_SH])
                nc.vector.tensor_copy(bcol[:, ng, :], tpb[:, 0:VH])
                nc.vector.tensor_copy(acol[:, ng, :], tpb[:, VH:BA_SH])

    # ---- qkvz projection: one (n, q) unit ----
    wt_cur = {}

    def emit_proj_unit(half, hT, n, q):
        kind, idx = tile_kind(n)
        if q == 0:
            wt = stW.tile([128, H // 128, 128], BF16, tag="wt", bufs=2,
                          name="wt")
            nc.scalar.dma_start(
                out=wt,
                in_=wq16d.rearrange("(i p) c -> p i c", p=128)[:, :, ts(n, 128)])
            wt_cur[0] = wt
        wt = wt_cur[0]
        if True:
            if True:
                pp = psP.tile([128, 512], F32, tag="pp")
                for i in range(H // 128):
                    nc.tensor.matmul(pp, wt[:, i, :], hT[:, i, ts(q, 512)],
                                     start=(i == 0), stop=(i == H // 128 - 1))
                ch0 = half * HCHUNK + q * 4
                if kind == "k":
                    dst = kdT[:, half, idx, ts(q, 512)]
                    evict_copy(dst, pp)
                    rowst = stR.tile([128, 4, 128], BF16, tag="rowst")
                    nc.scalar.dma_start(out=rowst, in_=dst, transpose=True)
                    scr = stR.tile([128, 4, 128], BF16, tag="nscr")
                    for b4 in range(4):
                        nc.scalar.activation(scr[:, b4, :], rowst[:, b4, :],
                                             AFT.Square,
                                             accum_out=normk[:, ch0 + b4, idx:idx + 1])
                    nc.scalar.dma_start(
                        out=bass.AP(
                            tensor=krowD.tensor,
                            offset=krowD.offset + ch0 * 128 * KH * 128 + idx * 128,
                            ap=[[KH * 128, 128], [128 * KH * 128, 4], [1, 128]]),
                        in_=rowst)
                elif kind == "q":
                    ev = stE.tile([128, 512], BF16, tag="ev")
                    evict_copy(ev, pp)
                    rowst = stR.tile([128, 4, 128], BF16, tag="rowst")
                    nc.scalar.dma_start(out=rowst, in_=ev, transpose=True)
                    scr = stR.tile([128, 4, 128], BF16, tag="nscr")
                    for b4 in range(4):
                        nc.scalar.activation(scr[:, b4, :], rowst[:, b4, :],
                                             AFT.Square,
                                             accum_out=normq[:, ch0 + b4, idx:idx + 1])
                    nc.scalar.dma_start(
                        out=bass.AP(
                            tensor=qrowD.tensor,
                            offset=qrowD.offset + ch0 * 128 * KH * 128 + idx * 128,
                            ap=[[KH * 128, 128], [128 * KH * 128, 4], [1, 128]]),
                        in_=rowst)
                elif kind == "v":
                    ev = stE.tile([128, 512], BF16, tag="ev")
                    evict_copy(ev, pp)
                    rowst = stR.tile([128, 4, 128], BF16, tag="rowst")
                    nc.scalar.dma_start(out=rowst, in_=ev, transpose=True)
                    nc.scalar.dma_start(
                        out=bass.AP(
                            tensor=vrowD.tensor,
                            offset=vrowD.offset + ch0 * 128 * VH * 128 + idx * 128,
                            ap=[[VH * 128, 128], [128 * VH * 128, 4], [1, 128]]),
                        in_=rowst)
                else:  # z -> silu -> szD
                    ev = stE.tile([128, 512], BF16, tag="ev")
                    evict_copy(ev, pp)
                    zrow = stZl.tile([128, 4, 128], BF16, tag="zrow")
                    nc.scalar.dma_start(out=zrow, in_=ev, transpose=True)
                    sgm = stZl.tile([128, 4, 128], BF16, tag="sgm")
                    nc.scalar.activation(sgm, zrow, AFT.Sigmoid)
                    zn = stZl.tile([128, 4, 128], BF16, tag="zn")
                    nc.vector.tensor_tensor(
                        zn, zrow, nwz[:, None, :].to_broadcast((128, 4, 128)),
                        op=ALU.mult)
                    szt = stZl.tile([128, 4, 128], BF16, tag="szt")
                    nc.vector.tensor_tensor(szt, zn, sgm, op=ALU.mult)
                    nc.scalar.dma_start(
                        out=bass.AP(
                            tensor=szD.tensor,
                            offset=szD.offset + ch0 * 128 * VH * 128 + idx * 128,
                            ap=[[VH * 128, 128], [128 * VH * 128, 4], [1, 128]]),
                        in_=szt)

    # =============== COLFORM PHASE (per half) ===============
    def emit_colform(half):
        hs = ds(half * HCHUNK, HCHUNK)
        nc.scalar.activation(beta_col[:, hs, :], bcol[:, hs, :], AFT.Sigmoid)
        nc.scalar.activation(lnb_col[:, hs, :], beta_col[:, hs, :], AFT.Ln)
        nc.vector.tensor_tensor(g_col[:, hs, :], acol[:, hs, :],
                                dtb_b.to_broadcast((128, HCHUNK, VH)), op=ALU.add)
        nc.scalar.activation(g_col[:, hs, :], g_col[:, hs, :], AFT.Exp)
        nc.scalar.activation(g_col[:, hs, :], g_col[:, hs, :], AFT.Ln, bias=1.0)
        nc.vector.tensor_tensor(g_col[:, hs, :], g_col[:, hs, :],
                                negea_b.to_broadcast((128, HCHUNK, VH)), op=ALU.mult)
        for j in range(HCHUNK):
            n = half * HCHUNK + j
            pc = psP.tile([128, VH], F32, tag="pp")
            nc.tensor.matmul(pc, uincl, g_col[:, n, :], start=True, stop=True)
            nc.vector.tensor_copy(c_col[:, n, :], pc)
        nc.scalar.activation(gam_col[:, hs, :], c_col[:, hs, :], AFT.Exp)
        nc.sync.dma_start(out=cendD[:, ds(half * HCHUNK * VH, HCHUNK * VH)],
                          in_=c_col[127:128, hs, :].rearrange("p a b -> p (a b)"))
        nc.sync.dma_start(
            out=cendb[:, hs, :],
            in_=prepend_bcast(cendD[0:1, ds(half * HCHUNK * VH, HCHUNK * VH)]
                              .rearrange("o (a b) -> o a b", b=VH)))
        nc.scalar.activation(gend_col[:, hs, :], cendb[:, hs, :], AFT.Exp)
        nc.vector.tensor_tensor(gi_col[:, hs, :], cendb[:, hs, :], c_col[:, hs, :],
                                op=ALU.subtract)
        nc.scalar.activation(gi_col[:, hs, :], gi_col[:, hs, :], AFT.Exp)
        # norm scales
        nc.scalar.activation(lnrk_col[:, hs, :], normk[:, hs, :], AFT.Ln, bias=c_eps)
        nc.vector.tensor_scalar_mul(lnrk_col[:, hs, :], lnrk_col[:, hs, :], -0.5)
        nc.scalar.activation(rkv_col[:, hs, :], lnrk_col[:, hs, :], AFT.Exp)
        nc.scalar.activation(lnrq_col[:, hs, :], normq[:, hs, :], AFT.Ln, bias=c_eps)
        nc.vector.tensor_scalar(lnrq_col[:, hs, :], lnrq_col[:, hs, :], -0.5,
                                -0.5 * LN128, op0=ALU.mult, op1=ALU.add)
        nc.scalar.activation(rqv_col[:, hs, :], lnrq_col[:, hs, :], AFT.Exp)
        # products
        nc.vector.tensor_tensor(t1_col[:, hs, :], beta_col[:, hs, :],
                                gam_col[:, hs, :], op=ALU.mult)  # beta*gamma
        for kh in range(KH):
            vs = ds(kh * 2, 2)
            rk_b = rkv_col[:, hs, kh:kh + 1].to_broadcast((128, HCHUNK, 2))
            rq_b = rqv_col[:, hs, kh:kh + 1].to_broadcast((128, HCHUNK, 2))
            nc.vector.tensor_tensor(sR1_col[:, hs, vs], t1_col[:, hs, vs], rk_b,
                                    op=ALU.mult)
            nc.vector.tensor_tensor(sK_col[:, hs, vs], gi_col[:, hs, vs], rk_b,
                                    op=ALU.mult)
            nc.vector.tensor_tensor(ogq_col[:, hs, vs], gam_col[:, hs, vs], rq_b,
                                    op=ALU.mult)
            lnrk_b = lnrk_col[:, hs, kh:kh + 1].to_broadcast((128, HCHUNK, 2))
            nc.vector.tensor_tensor(bek_col[:, hs, vs], lnrk_b, c_col[:, hs, vs],
                                    op=ALU.subtract)
        # chat/ctil rows
        nc.vector.tensor_tensor(t1_col[:, hs, :], c_col[:, hs, :],
                                lnb_col[:, hs, :], op=ALU.add)  # c + ln(beta)
        for vh in range(VH):
            kh = vh // 2
            nc.vector.tensor_tensor(bro_col[:, hs, 2 * vh], t1_col[:, hs, vh],
                                    lnrk_col[:, hs, kh], op=ALU.add)
            nc.vector.tensor_tensor(bro_col[:, hs, 2 * vh + 1], c_col[:, hs, vh],
                                    lnrq_col[:, hs, kh], op=ALU.add)
        # transpose bro -> browsD (per-chunk writes)
        for j in range(HCHUNK):
            n = half * HCHUNK + j
            pbr = psP.tile([2 * VH, 128], F32, tag="pp")
            nc.tensor.transpose(pbr, bro_col[:, n, :], ident)
            sbr = work.tile([2 * VH, 128], F32, tag="sbr")
            nc.vector.tensor_copy(sbr, pbr)
            nc.sync.dma_start(out=browsD[:, ts(n, 128)], in_=sbr)

    # =============== REC PHASE (per half) ===============
    xth = {}

    def emit_rec(half, tail_hooks=None, proj_units=None):
        xth[half] = hT_pool.tile([128, VH, HL], BF16, tag="xTh", name="xTh")

        for j in range(HCHUNK):
            n = half * HCHUNK + j
            csl = ds(j * 128, 128)
            kD = work.tile([128, KH, 128], BF16, tag="kD", bufs=2)
            nc.sync.dma_start(out=kD, in_=krowD[n])
            vD = work.tile([128, VH, 128], BF16, tag="vD", bufs=2)
            nc.sync.dma_start(out=vD, in_=vrowD[n])
            qD = work.tile([128, KH, 128], BF16, tag="qD", bufs=2)
            nc.sync.dma_start(out=qD, in_=qrowD[n])
            szv = work.tile([128, VH, 128], BF16, tag="szv", bufs=2)
            nc.sync.dma_start(out=szv, in_=szD[n])
            qTc = work.tile([128, KH, 128], BF16, tag="qTc", bufs=2)
            nc.sync.dma_start(out=qTc, in_=qD, transpose=True)
            chb = []
            for vh in range(VH):
                cb = work.tile([128, 2, 128], F32, tag=f"chb{vh}", bufs=2)
                nc.sync.dma_start(out=cb, in_=prepend_bcast(
                    browsD[ds(2 * vh, 2), ds(n * 128, 128)]))
                chb.append(cb)

            # q~ rows + transpose to dim-major
            qt_r = work.tile([128, VH, 128], BF16, tag="qt_r")
            for vh in range(VH):
                nc.vector.tensor_scalar(qt_r[:, vh, :], qD[:, vh // 2, :],
                                        ogq_col[:, n, vh:vh + 1], None,
                                        op0=ALU.mult)
            qtT = work.tile([128, VH, 128], BF16, tag="qtT")
            nc.scalar.dma_start(out=qtT, in_=qt_r, transpose=True)

            ag_t = [[None, None], [None, None]]
            Z4_t = []
            K4_t = []
            for kh in range(KH):
                psk = psK.tile([128, 2, 128], F32, tag="kkq")
                kT = kdT[:, half, kh, csl]
                nc.tensor.matmul(psk[:, 0, :], kT, kT, start=True, stop=True)
                nc.tensor.matmul(psk[:, 1, :], kT, qTc[:, kh, :],
                                 start=False, stop=True)

                for vi in range(2):
                    vh = kh * 2 + vi
                    # masked exponent: kept entries chat-ish, masked -> -5e4
                    r12 = work.tile([128, 2, 128], F32, tag="r12")
                    nc.vector.tensor_tensor(r12, chb[vh], maskADD, op=ALU.add)
                    # e12 = exp(r12 + lnrk_j - c_j)  (positive A | Ghat)
                    e12 = work.tile([128, 2, 128], F32, tag="e12")
                    nc.scalar.activation(e12, r12, AFT.Exp,
                                         bias=bek_col[:, n, vh:vh + 1])
                    ag = work.tile([128, 2, 128], BF16, tag="ag", bufs=6)
                    nc.vector.tensor_tensor(ag, e12, psk, op=ALU.mult)
                    ag_t[kh][vi] = ag

                # solve: Z = R - A^T R  (ag block0 = +A stored [j,i])
                R4 = work.tile([128, 2, 2, 128], BF16, tag="R4")
                for vi in range(2):
                    vh = kh * 2 + vi
                    nc.vector.tensor_scalar(R4[:, vi, 0, :], vD[:, vh, :],
                                            beta_col[:, n, vh:vh + 1], None,
                                            op0=ALU.mult)
                    nc.vector.tensor_scalar(R4[:, vi, 1, :], kD[:, kh, :],
                                            sR1_col[:, n, vh:vh + 1], None,
                                            op0=ALU.mult)
                psz = psZ.tile([128, 2, 256], F32, tag="pz")
                for vi in range(2):
                    nc.tensor.matmul(
                        psz[:, vi, :], ag_t[kh][vi][:, 0, :],
                        R4[:, vi, :, :].rearrange("p a b -> p (a b)"),
                        start=(vi == 0), stop=True)
                Z4 = work.tile([128, 2, 2, 128], BF16, tag="Z4", bufs=3)
                nc.vector.tensor_tensor(
                    Z4.rearrange("p a b c -> p (a b c)"),
                    R4.rearrange("p a b c -> p (a b c)"),
                    psz.rearrange("p a b -> p (a b)"), op=ALU.subtract)
                Z4_t.append(Z4)

                K4 = work.tile([128, 2, 128], BF16, tag="K4", bufs=3)
                for vi in range(2):
                    vh = kh * 2 + vi
                    nc.vector.tensor_scalar(K4[:, vi, :], kD[:, kh, :],
                                            sK_col[:, n, vh:vh + 1], None,
                                            op0=ALU.mult)
                K4_t.append(K4)

            # P / CM precompute + psO precompute
            QP_t = []
            Mst_t = []
            pso = psO.tile([128, VH, 128], F32, tag="psO")
            first_o = True
            for kh in range(KH):
                ppc = psPC.tile([128, 2, 2, 128], F32, tag="pc")
                for vi in range(2):
                    Wt = Z4_t[kh][:, vi, 1, :]
                    nc.tensor.matmul(ppc[:, vi, 0, :], Wt, ag_t[kh][vi][:, 1, :],
                                     start=(vi == 0), stop=False)
                    nc.tensor.matmul(ppc[:, vi, 1, :], Wt, K4_t[kh][:, vi, :],
                                     start=False, stop=True)
                QP2 = work.tile([128, 2, 128], BF16, tag="QP2", bufs=3)
                nc.vector.tensor_tensor(QP2, qtT[:, ds(kh * 2, 2), :],
                                        ppc[:, :, 0, :], op=ALU.subtract)
                Mst2 = work.tile([128, 2, 128], BF16, tag="Mst2", bufs=3)
                nc.scalar.activation(Mst2, ppc[:, :, 1, :], AFT.Copy, scale=-1.0)
                QP_t.append(QP2)
                Mst_t.append(Mst2)
                for vi in range(2):
                    vh = kh * 2 + vi
                    nc.tensor.matmul(pso[:, vh, :], ag_t[kh][vi][:, 1, :],
                                     Z4_t[kh][:, vi, 0, :],
                                     start=first_o, stop=False)
                    first_o = False

            # ---- sequential chain ----
            Sold = S4c[0]
            Sg4 = work.tile([128, VH, 128], BF16, tag="Sg4")
            for vh in range(VH):
                nc.vector.tensor_scalar(Sg4[:, vh, :], Sold[:, vh, :],
                                        gend_col[:, n, vh:vh + 1], None,
                                        op0=ALU.mult)
            pss = psS.tile([128, VH, 128], F32, tag="psS")
            for vh in range(VH):
                kh, vi = vh // 2, vh % 2
                nc.tensor.matmul(pss[:, vh, :], Mst_t[kh][:, vi, :],
                                 Sold[:, vh, :], start=(vh == 0), stop=False)
                nc.tensor.matmul(pss[:, vh, :], K4_t[kh][:, vi, :],
                                 Z4_t[kh][:, vi, 0, :], start=False, stop=True)
                nc.tensor.matmul(pso[:, vh, :], QP_t[kh][:, vi, :],
                                 Sold[:, vh, :], start=False, stop=True)
            Snew = spool.tile([128, VH, DV], BF16, tag="S4")
            nc.vector.tensor_tensor(
                Snew.rearrange("p a b -> p (a b)"),
                pss.rearrange("p a b -> p (a b)"),
                Sg4.rearrange("p a b -> p (a b)"), op=ALU.add)
            S4c[0] = Snew

            # ---- x output ----
            sqd = work.tile([128, 128], BF16, tag="sqd")
            for vh in range(VH):
                nc.scalar.activation(sqd, pso[:, vh, :], AFT.Square,
                                     accum_out=sscol[:, n, vh:vh + 1])
            rtmp = work.tile([128, VH], F32, tag="rtmp")
            nc.vector.tensor_scalar(rtmp, sscol[:, n, :], 1.0 / DV, EPS,
                                    op0=ALU.mult, op1=ALU.add)
            rtmp2 = work.tile([128, VH], F32, tag="rtmp2")
            nc.vector.reciprocal(rtmp2, rtmp)
            nc.scalar.activation(rstdc[:, n, :], rtmp2, AFT.Sqrt)
            szr = work.tile([128, VH, 128], BF16, tag="szr")
            xr = work.tile([128, VH, 128], BF16, tag="xr")
            for vh in range(VH):
                nc.scalar.activation(szr[:, vh, :], szv[:, vh, :], AFT.Copy,
                                     scale=rstdc[:, n, vh:vh + 1])
            nc.vector.tensor_tensor(
                xr.rearrange("p a b -> p (a b)"),
                pso.rearrange("p a b -> p (a b)"),
                szr.rearrange("p a b -> p (a b)"), op=ALU.mult)
            nc.scalar.dma_start(out=xth[half][:, :, ts(j, 128)], in_=xr,
                              transpose=True)
            if tail_hooks and j in tail_hooks:
                th_half, th_sg = tail_hooks[j]
                emit_tail_group(th_half, th_sg)
            if proj_units:
                for _ in range(3):
                    if proj_units:
                        proj_units.pop(0)()

    # =============== TAIL (out-projection, one 512-token group) ===============
    def emit_tail_group(half, sg):
        for nt in range(H // 128):
            po = psP.tile([128, 512], F32, tag="pp")
            for i in range(VH):
                nc.tensor.matmul(
                    po, wout_bf[:, i, ts(nt, 128)],
                    xth[half][:, i, ds(sg * 512, 512)],
                    start=(i == 0), stop=(i == VH - 1))
            ev = stE.tile([128, 512], BF16, tag="outev", bufs=2)
            evict_copy(ev, po, alternate=True)
            nc.gpsimd.dma_start(
                out=out[ts(nt, 128), ds(half * 2048 + sg * 512, 512)],
                in_=ev)

    # =============== emission order (pipeline priority) ===============
    hT0 = emit_proj_dmas(0)
    emit_ba_proj(0, hT0)
    for n in range(12):
        for q in range(4):
            emit_proj_unit(0, hT0, n, q)
    emit_colform(0)
    hT1 = emit_proj_dmas(1)
    units1 = [
        (lambda: emit_ba_proj(1, hT1))
    ] + [
        (lambda n=n, q=q: emit_proj_unit(1, hT1, n, q))
        for n in range(12) for q in range(4)
    ]
    emit_rec(0, proj_units=units1)
    while units1:
        units1.pop(0)()
    emit_colform(1)
    for sg in range(4):
        emit_tail_group(0, sg)
    hooks = {9: (1, 0), 11: (1, 1), 13: (1, 2)}
    emit_rec(1, tail_hooks=hooks)
    emit_tail_group(1, 3)

    ctx.close()
    return nc


_CACHED = None


def _build():
    global _CACHED
    if _CACHED is not None:
        return _CACHED
    nc = bacc.Bacc("TRN2", target_bir_lowering=False, debug=False)
    with tile.TileContext(nc) as tc:
        build_kernel(nc, tc)
    nc.compile()
    _CACHED = nc
    return nc


def make_in_maps(inputs):
    hidden = np.ascontiguousarray(np.asarray(inputs["hidden_states"], np.float32))
    W_qkvz = np.asarray(inputs["W_qkvz"], np.float32)
    W_ba = np.asarray(inputs["W_ba"], np.float32)
    A_log = np.asarray(inputs["A_log"], np.float32)
    dt_bias = np.asarray(inputs["dt_bias"], np.float32)
    norm_w = np.asarray(inputs["norm_weight"], np.float32)
    W_out = np.asarray(inputs["W_out"], np.float32)
    in_maps = []
    for c in range(NCORES):
        wba_sh = W_ba[:, c * BA_SH:(c + 1) * BA_SH]
        wba_r = wba_sh[:, [0, 1, 4, 5, 2, 3, 6, 7]]
        in_maps.append({
            "hidden": hidden,
            "wqkvz": np.ascontiguousarray(W_qkvz[:, c * QKVZ_SH:(c + 1) * QKVZ_SH]),
            "wba": np.ascontiguousarray(wba_r),
            "alog": np.ascontiguousarray(A_log[c * VH:(c + 1) * VH].reshape(1, VH)),
            "dtb": np.ascontiguousarray(dt_bias[c * VH:(c + 1) * VH].reshape(1, VH)),
            "nw": np.ascontiguousarray(norm_w.reshape(1, DV)),
            "wout": np.ascontiguousarray(W_out[c * VH * DV:(c + 1) * VH * DV, :]),
        })
    return in_maps


def kernel(**inputs) -> np.ndarray:
    from concourse import bass_utils

    nc = _build()
    in_maps = make_in_maps(inputs)
    res = bass_utils.run_bass_kernel_spmd(nc, in_maps, core_ids=list(range(NCORES)))
    total = None
    for r in res.results:
        o = np.asarray(r["out"], np.float32)
        total = o if total is None else total + o
    return np.ascontiguousarray(total.T)
